# revision 64
# baseline (speedup 1.0000x reference)
"""Trainium2 Bass kernel for nn_LocalizeAttention (27-point 3D neighbourhood gather).

out[b,h,(pi,pj,pk),(di,dj,dk),d] = x[b,h,(pi+di-1, pj+dj-1, pk+dk-1),d], zero outside.

All device data is bf16 (the correctness gate is rel_err < 2e-2; bf16
round-trip error is ~2^-9). The host casts x -> bf16 before upload and
upcasts the bf16 result -> f32 after download, halving the HBM traffic
of this DMA-bound kernel. The 0/1 shift matmuls and all copies are
value-exact in bf16, so the only error is the initial input rounding.

Strategy (per core, SPMD over 8 cores; 2 (b,h) volumes per core):
  - host zero-pads each volume to [26,26,26,32] bf16
  - partition rows = (v 2, pi 24, pjo 8) = 384 = 3 exact 128-partition tiles
    (pji=3 keeps all 16 SDMA engines evenly loaded and cuts per-partition
    copy work 25% vs a 96-partition layout)
  - per partition-tile, 3 slabs (one per di'); slab free dim = (pj 3-wide + 2
    halo, pk_padded 26, d 32) so dj'/dk' are free-dim offsets. Ptile 0 loads
    all 3 slabs from HBM; ptiles 1-2 load only the di'=0 slab + 2 extra rows
    and synthesize the di'=1,2 partition-shifted slabs on the tensor engine
    (matmul with 0/1 shift matrices, PSUM drained by vector/scalar) - this
    removes 2/3 of the HBM read traffic
  - 9 shifted copies per output tile assemble [128, (pjl 3, pkl 4, s 27,
    d 32)]; the 3 dk' merge into one contiguous 96-element run; whole
    otiles go to Vector or Scalar by greedy list scheduling (GpSimd is too
    slow in bf16 and its otile store stalled the ring every transition)
  - slabs triple-buffered across partition-tiles so the k==0 dense
    prefetch for T+1 carries no WAR wait; 5 rotating output buffers;
    all DMAs on the sync HWDGE ring (a second HWDGE ring exists on the
    scalar engine, but DMA issues there queue behind its copy workload -
    only the startup halo/W loads use it, issuing in parallel with the
    sync ring's dense issues while both queues are empty); stores emitted
    in predicted-finish order; pjo=7 halo fixup DMAs (WAW on the psum
    drains) deferred to each ptile's last store slot; load descriptors
    split so each DMA spans all 16 SDMA engines

Measured on trn2 (8 cores): ~154us fast-mode, ~170us when chip-level HBM
contention phase-aligns badly; chip roofline for 8x53.1MB is ~146us.
"""

import numpy as np

B, HEADS, DH = 2, 8, 32
H = W = D = 24
N = H * W * D
FN = 27
NCORES = 8
NVOL = (B * HEADS) // NCORES  # 2 volumes per core
USE_PE = True                 # tensor-engine shift for ptiles 1-2

# copy-engine speed estimates (ns per bf16 element per partition), refined
# from trace measurements; used only for load balancing / emission order.
# GpSimd is dropped: at ~3.5 ns/el its one otile per ptile became a 36us
# straggler whose store stalled the FIFO sync ring at every ptile
# transition; V+A alone outproduce the DMA ring comfortably at pkb=4.
RATES = {"V": 0.64, "A": 1.08}


def _ptile_segments():
    """3 partition-tiles of 128 rows; rows = (v, pi, pjo) with pjo in [0,8).
    Returns per tile a list of (v, pi0, npi, p0, pstep) segments: rows of the
    segment occupy partitions p0, p0+pstep, ... For the volume-spanning tile
    the two segments are interleaved (pstep=2) so every DMA covers all 16
    SDMA engines instead of starving half of them."""
    return [
        [(0, 0, 16, 0, 1)],
        [(0, 16, 8, 0, 2), (1, 0, 8, 1, 2)],
        [(1, 8, 16, 0, 1)],
    ]


def _layout_of(T):
    # 0 = stride-1 single-volume tile, 1 = volume-interleaved tile
    return 1 if T == 1 else 0


def build_w():
    """Shift matrices for the PE path. idx 0-3: slab_di[:valid] = W.T @ slab0
    (idx = layout*2 + di-1); tail partitions are DMA-loaded directly.
    idx 4+L: halo synthesis - halo[p] = dense[p+pstep][0:halo_f] for pjo<7
    (pjo=7 partitions get zeros; their real first row is DMA-loaded)."""
    w = np.zeros((6, 128, 128), dtype=np.float32)
    for L in (0, 1):
        for di in (1, 2):
            step = 8 * di if L == 0 else 16 * di
            for p in range(128 - step):
                w[L * 2 + (di - 1), p + step, p] = 1.0
        pstep = 1 if L == 0 else 2
        for p in range(128):
            pjo = (p % 8) if L == 0 else ((p >> 1) % 8)
            if pjo < 7:
                w[4 + L, p + pstep, p] = 1.0
    return w


def tail_segs(T, di):
    """Shifted-tail rows DMA-loaded directly into slab_di for ptile T:
    list of (v, pi_pad_row0, nrow, pdst, pstep); each row spans 8 pjo."""
    if T == 1:
        if di == 1:
            return [(0, 24, 1, 112, 2), (1, 8, 1, 113, 2)]
        return [(0, 24, 2, 96, 2), (1, 8, 2, 97, 2)]
    if T == 2:
        if di == 1:
            return [(1, 24, 1, 120, 1)]
        return [(1, 24, 2, 112, 1)]
    return []


def _build_nc(nvol, pkb=4, nobuf=5, rates=RATES, mixed="tail", nset=3,
              dense_at=0, shifts_at=2, merged=False, prefetch2=False,
              t0split=2):
    import concourse.bass as bass
    import concourse.mybir as mybir
    from concourse.ap import AP
    from concourse.bacc import Bacc
    from concourse.tile import TileContext

    f32 = mybir.dt.float32
    bf16 = mybir.dt.bfloat16
    dh = DH
    hp = wp = dp = 26
    pji, pjo = 3, 8
    P = 128
    fn = FN
    s_jp = dp * dh                 # 832: xpad pj stride (elements)
    s_ip = wp * s_jp               # 21632: xpad pi stride
    vol_pad = hp * s_ip
    slab_f = (pji + 2) * s_jp      # 4160
    out_f = pji * pkb * fn * dh    # otile free size
    run = 3 * dh                   # merged (dk', d) contiguous run
    vol_out = N * fn * dh

    def blocks_of(T):
        # (pk0, pkb_i) output blocks for ptile T. The first blocks of T0
        # are halved so the first store issues ~2.5us sooner (the startup
        # is copy-latency-bound); the last blocks of the final ptile are
        # halved so the unoverlapped final store drain is shorter.
        full = [(k, pkb) for k in range(0, D, pkb)]
        if not mixed:
            return full
        if T == 0 and mixed != "tail":
            return [(0, pkb // 2), (pkb // 2, pkb - pkb // 2)] + full[1:]
        if T == NT - 1:
            return full[:-1] + [(D - pkb, pkb // 2),
                                (D - pkb + pkb // 2, pkb - pkb // 2)]
        return full
    row_out = pji * D * fn * dh    # 62208: output elems per partition row
    pjl_out = D * fn * dh          # 20736: output elems per pj line
    segs = _ptile_segments()
    NT = len(segs)

    nc = Bacc()
    xpad = nc.declare_dram_parameter("xpad", [nvol, hp, wp, dp, dh], bf16,
                                     isOutput=False)
    if USE_PE:
        # k-major layout: one contiguous run per partition row
        wsh = nc.declare_dram_parameter("wsh", [128, 6, 128], bf16,
                                        isOutput=False)
    out = nc.declare_dram_parameter("out", [nvol, N, fn, dh], bf16,
                                    isOutput=True)
    xt = xpad[:].tensor
    ot = out[:].tensor

    NSET = nset
    import contextlib
    from types import SimpleNamespace
    with contextlib.ExitStack() as ctx:
        tc = ctx.enter_context(TileContext(nc))
        # slabs: 3 sets so the prefetch load for ptile T+1 targets the set
        # last read by T-2 (already drained) -> the load DMA never carries
        # a WAR wait and can't stall whichever ring issues it.
        # merged=True (3 copies/otile with di as a 4th free dim) does NOT
        # compile: neuronxcc codegen only has TPB_TENSOR3D patterns, so
        # engine copies are limited to partition + 3 free dims. Kept for
        # documentation; do not enable.
        if merged:
            slabsets = [ctx.enter_context(
                nc.sbuf_tensor(f"slabs{s}", [P, 3 * slab_f], bf16))
                for s in range(NSET)]
            srow = 3 * slab_f     # partition stride of a slab tensor row

            def sl(T, di):
                h = slabsets[T % NSET][:]
                return SimpleNamespace(tensor=h.tensor,
                                       offset=h.offset + di * slab_f)
        else:
            slabs = [[ctx.enter_context(
                nc.sbuf_tensor(f"slab{s}_{i}", [P, slab_f], bf16))
                for i in range(3)] for s in range(NSET)]
            srow = slab_f

            def sl(T, di):
                return slabs[T % NSET][di][:]
        otiles = [ctx.enter_context(nc.sbuf_tensor(f"otile{i}", [P, out_f],
                                                   bf16))
                  for i in range(nobuf)]
        scratch = ctx.enter_context(nc.sbuf_tensor("scratch", [P, 32], bf16))
        if USE_PE:
            wsb = ctx.enter_context(nc.sbuf_tensor("wsb", [P, 6 * 128], bf16))
            psums = [ctx.enter_context(nc.psum_tensor(f"ps{i}", [P, 512], f32))
                     for i in range(2)]

        dense_f = pji * s_jp           # 2496: non-overlapping window part
        halo_f = slab_f - dense_f      # 1664: 2-row halo (re-read)

        # all mid-stream DMAs stay on the sync HWDGE ring. The scalar ring
        # regressed badly: its DMA_DIRECT2D issues queue behind the scalar
        # engine's copy workload, so prefetch loads for T+1 only issued
        # near the end of T and T+1's copies stalled on them. It is used
        # only for the startup halo/W loads, where scalar's queue is empty
        # and the issue runs parallel to the sync ring's dense issues.
        def emit_dense(T, di, ring=None):
            # dense (non-overlapping) window part: descriptors sweep DRAM
            # sequentially and are split so every DMA carries >=256
            # descriptors (descriptor blocks then cover all 16 SDMA engines)
            ring = ring or nc.sync
            slab = sl(T, di)
            for (v, pi0, npi, p0, pstep) in segs[T]:
                base = v * vol_pad + (pi0 + di) * s_ip
                ns = 2 if npi * pjo >= 128 else 4
                src = AP(xt, base,
                         [[s_ip, npi], [dense_f // ns, ns * pjo],
                          [1, dense_f // ns]])
                dst = AP(slab.tensor, slab.offset + p0 * srow,
                         [[pstep * srow, npi * pjo], [dense_f // ns, ns],
                          [1, dense_f // ns]])
                ring.dma_start(out=dst, in_=src)

        def emit_halo(T, di, ring=None):
            # overlapping 2-row halo: only 128 descriptors -> lands on half
            # the SDMA engines; unavoidable within the 3-dim DMA AP limit
            # (SWDGE issue was tried and regressed: Q7 generation cost)
            ring = ring or nc.sync
            slab = sl(T, di)
            for (v, pi0, npi, p0, pstep) in segs[T]:
                base = v * vol_pad + (pi0 + di) * s_ip
                src = AP(xt, base + dense_f,
                         [[s_ip, npi], [dense_f, pjo], [1, halo_f]])
                dst = AP(slab.tensor, slab.offset + p0 * srow + dense_f,
                         [[pstep * srow, npi * pjo], [1, halo_f]])
                ring.dma_start(out=dst, in_=src)

        def emit_loads(T, di):
            # startup: dense on the (empty) sync ring, halo on scalar, so
            # the serial ~0.65us-per-DMA issue cost runs on both rings
            emit_dense(T, di, ring=nc.sync)
            emit_halo(T, di, ring=nc.scalar)

        def tail_dmas(T):
            # shifted-tail rows that the W matmul cannot source from slab0;
            # one DMA per pi row with an 8-way free split (64 descriptors
            # instead of 8, spreading across more SDMA engines), returned as
            # (dst, src) pairs so each tiny DMA gets its own emission point
            sp = slab_f // 16
            out = []
            for di in (1, 2):
                slab = sl(T, di)
                for (v, row0, nrow, pdst, pstep) in tail_segs(T, di):
                    for r in range(nrow):
                        base = v * vol_pad + (row0 + r) * s_ip
                        src = AP(xt, base,
                                 [[dense_f, pjo], [sp, 16], [1, sp]])
                        dst = AP(slab.tensor,
                                 slab.offset + (pdst + r * pstep * pjo)
                                 * srow,
                                 [[pstep * srow, pjo], [sp, 16], [1, sp]])
                        out.append((dst, src))
            return out

        engs = {"V": nc.vector, "A": nc.scalar}

        def copy(ename, dst_ap, src_ap):
            e = engs[ename]
            if hasattr(e, "tensor_copy"):
                e.tensor_copy(out=dst_ap, in_=src_ap)
            else:
                e.copy(out=dst_ap, in_=src_ap)

        def emit_halo_synth(T, di, ci, fixups):
            # synthesize slab_di's halo from its dense part (pjo+1 partition
            # shift on PE); pjo=7 partitions get zeros from the matmul and
            # their single real row (pj_pad 24) is DMA-loaded over them
            L = _layout_of(T)
            slab = sl(T, di)
            wh = AP(wsb[:].tensor, wsb[:].offset + (4 + L) * 128,
                    [[6 * 128, 128], [1, 128]])
            for c0 in range(0, halo_f, 512):
                cw = min(512, halo_f - c0)
                ps = psums[ci % 2][:]
                pap = AP(ps.tensor, ps.offset, [[512, 128], [1, cw]])
                nc.tensor.matmul(
                    pap, wh,
                    AP(slab.tensor, slab.offset + c0,
                       [[srow, 128], [1, cw]]),
                    start=True, stop=True)
                copy("V" if ci % 2 else "A",
                     AP(slab.tensor, slab.offset + dense_f + c0,
                        [[srow, 128], [1, cw]]),
                     pap)
                ci += 1
            fixups.extend(fixup_dmas(T, di))
            return ci

        def fixup_dmas(T, di):
            # pjo=7 halo rows (single real row pj_pad 24) loaded directly
            # from DRAM over the zeros the matmuls leave there. Emitted by
            # the caller in the LAST store slot: the DMA carries a WAW wait
            # on the psum drains, so issuing it earlier would head-of-line
            # block the sync ring. Because the fixup lands AFTER the di=1,2
            # shift matmuls read slab0's halo, those matmuls see zeros in
            # pjo=7 rows — so slabs 1,2 get their own direct fixups too
            # instead of inheriting slab0's via the shift.
            slab = sl(T, di)
            hq = s_jp // 4
            out = []
            for (v, pi0, npi, p0, pstep) in segs[T]:
                src_ = AP(xt, v * vol_pad + (pi0 + di) * s_ip + 24 * s_jp,
                          [[s_ip, npi], [hq, 4], [1, hq]])
                dst_ = AP(slab.tensor,
                          slab.offset + (p0 + pstep * 7) * srow + dense_f,
                          [[pstep * 8 * srow, npi], [hq, 4], [1, hq]])
                out.append((dst_, src_))
            return out

        def emit_shifts(T):
            # synthesize slabs[T%NSET][1], [2] = partition-shifted slab0 on
            # PE; only the `valid` partition prefix is computed (tail rows
            # are DMA-loaded directly), halving the matmul work
            L = _layout_of(T)
            slab0 = sl(T, 0)
            fixups = []
            ci = emit_halo_synth(T, 0, 0, fixups)
            for di in (1, 2):
                valid = 128 - (8 if L == 0 else 16) * di
                dst_t = sl(T, di)
                w1 = AP(wsb[:].tensor,
                        wsb[:].offset + (L * 2 + (di - 1)) * 128,
                        [[6 * 128, 128], [1, valid]])
                for c0 in range(0, slab_f, 512):
                    cw = min(512, slab_f - c0)
                    ps = psums[ci % 2][:]
                    pap = AP(ps.tensor, ps.offset, [[512, valid], [1, cw]])
                    nc.tensor.matmul(
                        pap, w1,
                        AP(slab0.tensor, slab0.offset + c0,
                           [[srow, 128], [1, cw]]),
                        start=True, stop=True)
                    copy("V" if ci % 2 else "A",
                         AP(dst_t.tensor, dst_t.offset + c0,
                            [[srow, valid], [1, cw]]),
                         pap)
                    ci += 1
                fixups.extend(fixup_dmas(T, di))
            return fixups

        # greedy schedule: per partition-tile, assign pk-blocks to engines by
        # earliest predicted finish; emit in predicted-finish order
        clock = {k: 0.0 for k in rates}
        tix = 0
        if USE_PE:
            wt = wsh[:].tensor
            wdst = AP(wsb[:].tensor, wsb[:].offset,
                      [[6 * 128, 128], [1, 6 * 128]])
            wsrc = AP(wt, 0, [[6 * 128, 128], [1, 6 * 128]])
            nc.scalar.dma_start(out=wdst, in_=wsrc)
        for di in range(3):
            emit_loads(0, di)
        for T in range(NT):
            # wait-absorbers: soak the slab-DMA waits on each copy engine
            # (read one element from both the dense and halo regions)
            for ei, ename in enumerate(engs):
                for di in range(3):
                    slab = sl(T, di)
                    col = (ei * 3 + di) * 2
                    copy(ename,
                         AP(scratch[:].tensor, scratch[:].offset + col,
                            [[32, P], [1, 2]]),
                         AP(slab.tensor, slab.offset,
                            [[srow, P], [dense_f, 2]]))
            blocks = blocks_of(T)
            nblk = len(blocks)
            sched = []
            for t, (pk0, pkbi) in enumerate(blocks):
                cost = {e: rates[e] * pji * pkbi * run * 9 for e in rates}
                ename = min(rates, key=lambda k: clock[k] + cost[k])
                clock[ename] += cost[ename]
                sched.append((clock[ename], ename, pk0, pkbi))
            sched.sort()
            if prefetch2:
                # all wait-free loads for EVERY later ptile are emitted
                # during T0, filling the DMA idle of the copy-latency-bound
                # ramp and removing load work from the saturated mid-stream
                tails = []
                if T == 0:
                    for Tn in range(1, NT):
                        tails.extend(tail_dmas(Tn))
            else:
                tails = tail_dmas(T + 1) if (USE_PE and T + 1 < NT) else []
            # slot 0: dense prefetch; slot 2: PE shifts; last slot: fixups
            # (WAW-waiting on psum drains); tails spread over what remains
            tslots = [k for k in range(nblk - 1)
                      if k not in (dense_at, shifts_at)] or [1]
            fixups = []
            per = -(-len(tails) // len(tslots)) if tails else 0
            for k, (_fin, ename, pk0, pkbi) in enumerate(sched):
                if T + 1 < NT:
                    if USE_PE:
                        if k == dense_at:
                            if prefetch2 and T == 0:
                                for Tn in range(1, NT):
                                    emit_dense(Tn, 0)
                            elif not prefetch2:
                                emit_dense(T + 1, 0)
                        elif k == shifts_at:
                            fixups = emit_shifts(T + 1)
                        elif k == nblk - 1:
                            for dst, src in fixups:
                                nc.sync.dma_start(out=dst, in_=src)
                        elif per and k in tslots:
                            i = tslots.index(k)
                            for dst, src in tails[i * per:(i + 1) * per]:
                                nc.sync.dma_start(out=dst, in_=src)
                    elif k in (1, 3, 5):
                        emit_loads(T + 1, k // 2)
                otile = otiles[tix % nobuf][:]
                tix += 1
                ofree = pji * pkbi * fn * dh
                if merged:
                    # one copy per dj: di becomes a 4th free dim (stride
                    # slab_f on the src, 9*dh in the s-slot space on dst)
                    slab = sl(T, 0)
                    for dj in range(3):
                        src = AP(slab.tensor,
                                 slab.offset + dj * s_jp + pk0 * dh,
                                 [[srow, P], [slab_f, 3], [s_jp, pji],
                                  [dh, pkbi], [1, run]])
                        dst = AP(otile.tensor,
                                 otile.offset + dj * 3 * dh,
                                 [[out_f, P], [9 * dh, 3],
                                  [pkbi * fn * dh, pji], [fn * dh, pkbi],
                                  [1, run]])
                        copy(ename, dst, src)
                else:
                    # the first t0split otiles of T0 are split across both
                    # engines (V 6 / A 3 by measured rate) - the startup is
                    # bound on the first otile's copy latency, not
                    # throughput, and both engines are otherwise idle there
                    spl = T == 0 and k < t0split
                    for di in range(3):
                        slab = sl(T, di)
                        for dj in range(3):
                            src = AP(slab.tensor,
                                     slab.offset + dj * s_jp + pk0 * dh,
                                     [[srow, P], [s_jp, pji], [dh, pkbi],
                                      [1, run]])
                            dst = AP(otile.tensor,
                                     otile.offset + (di * 9 + dj * 3) * dh,
                                     [[out_f, P], [pkbi * fn * dh, pji],
                                      [fn * dh, pkbi], [1, run]])
                            en = ("V" if di < 2 else "A") if spl else ename
                            copy(en, dst, src)
                for (v, pi0, npi, p0, pstep) in segs[T]:
                    nrows = npi * pjo
                    sdst = AP(ot, v * vol_out + pi0 * pjo * row_out
                              + pk0 * fn * dh,
                              [[row_out, nrows], [pjl_out, pji],
                               [1, pkbi * fn * dh]])
                    ssrc = AP(otile.tensor, otile.offset + p0 * out_f,
                              [[pstep * out_f, nrows], [1, ofree]])
                    nc.sync.dma_start(out=sdst, in_=ssrc)

    nc.finalize()
    return nc


def _bf16():
    import ml_dtypes
    return ml_dtypes.bfloat16


def _pad_volumes(x):
    # x: [nvol, N, dh] f32 -> [nvol, hp, wp, dp, dh] bf16 zero-padded
    nvol = x.shape[0]
    xv = x.reshape(nvol, H, W, D, DH)
    xp = np.zeros((nvol, H + 2, W + 2, D + 2, DH), dtype=_bf16())
    xp[:, 1:H + 1, 1:W + 1, 1:D + 1, :] = xv.astype(_bf16())
    return xp


def _run(x, trace=False):
    import sys
    if "/opt/trn_rl_repo" not in sys.path:
        sys.path.insert(0, "/opt/trn_rl_repo")
    from concourse.bass_utils import run_bass_kernel_spmd

    x = np.asarray(x, dtype=np.float32)
    assert x.shape == (B, HEADS, N, DH), x.shape
    xf = x.reshape(B * HEADS, N, DH)
    nc = _build_nc(NVOL)
    w = None
    if USE_PE:
        # transpose to k-major [128, 6, 128] to match the DRAM declaration
        w = np.ascontiguousarray(
            build_w().transpose(1, 0, 2).astype(_bf16()))
    in_maps = []
    for i in range(NCORES):
        m = {"xpad": _pad_volumes(xf[i * NVOL:(i + 1) * NVOL])}
        if USE_PE:
            m["wsh"] = w
        in_maps.append(m)
    res = run_bass_kernel_spmd(nc, in_maps, list(range(NCORES)), trace=trace)
    outs = np.concatenate(
        [np.asarray(res.results[i]["out"]).astype(np.float32)
         for i in range(NCORES)], axis=0)
    return outs.reshape(B, HEADS, N, FN, DH), res


def kernel(x, height, width, depth, **_):
    assert int(height) == H and int(width) == W and int(depth) == D
    out, _res = _run(x, trace=False)
    return out


def kernel_profiled(x):
    out, res = _run(x, trace=True)
    return out, res

